# revision 27
# baseline (speedup 1.0000x reference)
"""Trainium2 Bass kernel for the hex-board pattern one-hot encoder.

Reference semantics: boards (B, 11, 11) in {-1,0,1} -> out (B, 27, 12, 12)
f32 where out[b,p,i,j] = 1 iff the 3-tuple (P[i,j], P[i,j+1], P[i+1,j]) of
the border-padded 13x13 board equals pattern p (patterns =
product([-1,0,1], repeat=3)), with wildcard corners at (0,0) [elem0],
(0,11) [elem1], (11,0) [elem2].

Final design, ~76-80us HW exec across runs (f32-output v1: ~188us; u8
board-major v2/v3: ~88us): output values are exactly {0, 1}, so the
device computes and writes the full one-hot tensor in UINT8 (127 MB
instead of 510 MB of HBM writes; the f32 materialization is a pure
dtype cast after the gather). Compute is the bottleneck, split across
both engines:
  - the idx chain runs in bf16 in three DVE ops: a fused
    scalar_tensor_tensor tmp = 3*P[g+1] + P[g+13] (walrus limits STT to
    2D/3D, so the rest is split), jb = 9*P[g] (4x tensor_scalar), and a
    4D tensor_tensor add that also compacts to the 12x12 subgrids; the
    +13 of the classic 0..26 code is folded into the compare constants
    (idx in -13..13),
  - 17 planes on VectorE as (idx == p-13) bf16->u8 tensor_scalar (2x),
  - 10 planes on ScalarE in ONE op each: u8(Derivative_Erf(idx+13-p)) =
    u8(1.1284*exp(-(idx-(p-13))^2)) which rounds to exactly 1 at equality
    and 0 otherwise (HW-validated; replaces the 2-op Square+Relu pair).

Layout is PLANE-MAJOR per macrotile, in SBUF and HBM ([27, s, 144] per
partition): every plane compare writes one dense 1-free-dim region, and
every plane-chunk store is one contiguous multi-KB run per partition.
(Board-major chunk stores produced 720-864 B strided DMA transfers whose
descriptor overhead halved effective HBM write bandwidth — measured
~200 GB/s/core vs ~340 with contiguous rows.) The host de-transposes
tiles during the u8 gather.

The padded board grid is shipped pre-cast as bf16 (ml_dtypes) so ScalarE
spends no time on int8->f32 casts and the chain needs no ScalarE at all.
Macrotile sizes (16, 14, 2): EVEN sizes only — the DVE 2x two-port mode
silently degrades to 1x when the compare's free size is odd (measured);
the tiny LAST tile bounds the end-of-kernel DMA drain. The idx chain for
tile m+1 runs on DVE before the plane compares of tile m so ScalarE
never stalls on idx.

The 3 wildcard corners need 2 extra ones each at a fixed position whose
value depends only on one board cell (or nothing at all for corner
(0,0)); a single idx value cannot make 3 planes fire, so those 6
fixed-position writes are applied on the host during the u8 gather
(0.15% of output elements).

Pure data parallel across 8 NeuronCores (batch sharding).
"""

import numpy as np

import concourse.bacc as bacc
import concourse.mybir as mybir
from concourse.mybir import AluOpType
from concourse.tile import TileContext

N_CORES = 8
BATCH = 32768
B_CORE = BATCH // N_CORES  # 4096
NPART = 128
BPP = B_CORE // NPART  # 32 boards per partition
SIZES = (16, 14, 2)  # boards/partition per macrotile; sum == BPP; all EVEN
PADW = BPP * 169 + 14  # flat padded boards per partition + shift tail
QF = 27 * 144  # output elems per board

F32 = mybir.dt.float32
BF16 = mybir.dt.bfloat16
U8 = mybir.dt.uint8

# plane split: 12 planes as VectorE is_equal bf16->u8 (2x two-port mode),
# 10 planes as ScalarE Derivative_Erf (1 op each), and 5 planes as VectorE
# is_equal bf16->bf16 (4x packed mode) into a separate bf16 output tensor
# that the host casts during the gather. The bf16 planes cost 2x the HBM
# bytes but nearly halve their DVE time; they are computed FIRST each tile
# so their larger stores issue early instead of in the drain tail.
N_U8 = 12  # planes [0, 12) on VectorE, u8
ACT_PS = list(range(12, 22))  # planes [12, 22) on ScalarE, u8
BF_PS = list(range(22, 27))  # planes [22, 27) on VectorE, bf16
NBF = len(BF_PS)


def build_nc(sizes=SIZES, debug=False):
    bpp = sum(sizes)
    padw = bpp * 169 + 14
    nm = len(sizes)
    offs = [sum(sizes[:i]) for i in range(nm)]

    nc = bacc.Bacc(
        "TRN2", target_bir_lowering=False, debug=debug, enable_partition_id=False
    )

    # board b_local = r*bpp + j (partition-major); input row per partition
    # is the bpp host-padded 169-elem 13x13 grids, pre-cast bf16.
    boards_h = nc.dram_tensor("boards", [NPART, padw], BF16, kind="ExternalInput")
    # HBM outputs are plane-major per tile: u8 [tile][22][s][144] for planes
    # 0..21 and bf16 [tile][5][s][144] for planes 22..26, per partition.
    out_h = nc.dram_tensor(
        "out", [NPART, bpp * 22 * 144], U8, kind="ExternalOutput"
    )
    outb_h = nc.dram_tensor(
        "outb", [NPART, bpp * NBF * 144], BF16, kind="ExternalOutput"
    )

    with TileContext(nc) as tc:
        with (
            tc.tile_pool(name="cpool", bufs=1) as cpool,
            tc.tile_pool(name="pfpool", bufs=3) as pfpool,
            tc.tile_pool(name="gpool", bufs=2) as gpool,
            tc.tile_pool(name="ipool", bufs=2) as ipool,
            tc.tile_pool(name="opool", bufs=2) as opool,
        ):
            # per-partition bias constants for the ScalarE dErf (13 - p),
            # built on ScalarE itself via Copy(scale=0, bias=...).
            negp = cpool.tile([NPART, 27], F32, name="negp")

            def negp_init():
                zsrc = nc.const_aps.tensor(0.0, [NPART, 1], F32)
                for p in ACT_PS:
                    nc.scalar.activation(
                        negp[:, p : p + 1], zsrc,
                        mybir.ActivationFunctionType.Copy,
                        bias=float(13 - p), scale=0.0,
                    )

            pf_tiles, idx_tiles = {}, {}

            def fetch(mi):
                if mi < nm and mi not in pf_tiles:
                    s = sizes[mi]
                    w = s * 169 + 14
                    g0 = offs[mi] * 169
                    Pf = pfpool.tile([NPART, w], BF16, name="Pf")
                    nc.sync.dma_start(out=Pf, in_=boards_h[:, g0 : g0 + w])
                    pf_tiles[mi] = Pf

            def chain(mi):
                """idx[mi] = 9*P[g] + 3*P[g+1] + P[g+13] (range -13..13) on
                the 12x12 subgrids, via two fused ops."""
                if mi >= nm or mi in idx_tiles:
                    return
                s = sizes[mi]
                ng = s * 169
                Pf = pf_tiles[mi]
                ib = gpool.tile([NPART, ng], BF16, name="ib")
                jb = gpool.tile([NPART, ng], BF16, name="jb")
                idx = ipool.tile([NPART, s, 144], BF16, name="idx")
                ibv = ib.rearrange("p (t a b) -> p t a b", a=13, b=13)
                jbv = jb.rearrange("p (t a b) -> p t a b", a=13, b=13)
                idxv4 = idx.rearrange("p t (a b) -> p t a b", a=12, b=12)
                # ib = (P[g+1] * 3) + P[g+13]  (fused; 2D so walrus allows STT)
                nc.vector.scalar_tensor_tensor(
                    ib, Pf[:, 1 : ng + 1], 3.0, Pf[:, 13 : ng + 13],
                    AluOpType.mult, AluOpType.add,
                )
                # jb = 9*P[g] (4x); idx = jb + ib on the 12x12 subgrids (STT
                # rejects 4D inputs, so scale separately then add via 4D TT)
                nc.vector.tensor_scalar(
                    jb, Pf[:, 0:ng], 9.0, None, AluOpType.mult
                )
                nc.vector.tensor_tensor(
                    idxv4, jbv[:, :, 0:12, 0:12], ibv[:, :, 0:12, 0:12],
                    AluOpType.add,
                )
                idx_tiles[mi] = idx

            for mi in range(nm):
                fetch(mi)
            negp_init()
            chain(0)

            for m in range(nm):
                s = sizes[m]
                idx = idx_tiles[m]
                # plane-major SBUF tiles: u8 [22][s][144], bf16 [5][s][144]
                out_t = opool.tile([NPART, 22, s, 144], U8, name="out_t")
                out_b = opool.tile([NPART, NBF, s, 144], BF16, name="out_b")
                ohv = out_h[:, offs[m] * 22 * 144 :][
                    :, : 22 * s * 144
                ].rearrange("p (q t f) -> p q t f", q=22, t=s, f=144)
                obv = outb_h[:, offs[m] * NBF * 144 :][
                    :, : NBF * s * 144
                ].rearrange("p (q t f) -> p q t f", q=NBF, t=s, f=144)

                # claim out_t's DMA WAR dep on ScalarE with a tiny op; its
                # own dErf overwrites it below.
                c0 = ACT_PS[0]
                nc.scalar.mul(out_t[:, c0, :, 0], out_t[:, c0, :, 0], 0.0)
                chain(m + 1)

                last = m == nm - 1
                dve_chunks = ((0, 6), (6, 12))
                act_chunks = ((12, 22),) if last else ((12, 17), (17, 22))
                # NB: keep the DMA trigger counts/rings exactly as measured
                # best — adding store chunks or moving them between rings
                # (Sync/GpSimd/Scalar) measured 3-5us SLOWER via DMAHW
                # completion-lane reshuffling, despite idle queues.

                idxf = idx.rearrange("p t f -> p (t f)")
                # flat 2D views: plane slices are contiguous in the
                # plane-major layout.
                sf = s * 144
                of = out_t.rearrange("p q t f -> p (q t f)")
                ob = out_b.rearrange("p q t f -> p (q t f)")
                # bf16 planes first (4x mode), one early store for all 5
                for k, p in enumerate(BF_PS):
                    nc.vector.tensor_scalar(
                        ob[:, k * sf : (k + 1) * sf], idxf,
                        float(p - 13), None, AluOpType.is_equal,
                    )
                nc.sync.dma_start(out=obv, in_=out_b)
                for a, b in dve_chunks:
                    for p in range(a, b):
                        nc.vector.tensor_scalar(
                            of[:, p * sf : (p + 1) * sf], idxf,
                            float(p - 13), None, AluOpType.is_equal,
                        )
                    nc.sync.dma_start(
                        out=ohv[:, a:b, :, :], in_=out_t[:, a:b, :, :]
                    )
                for a, b in act_chunks:
                    for p in range(a, b):
                        nc.scalar.activation(
                            of[:, p * sf : (p + 1) * sf], idxf,
                            mybir.ActivationFunctionType.Derivative_Erf,
                            bias=negp[:, p : p + 1], scale=1.0,
                        )
                    nc.scalar.dma_start(
                        out=ohv[:, a:b, :, :], in_=out_t[:, a:b, :, :]
                    )

    nc.finalize()
    return nc


def prep_core_input(boards_core, bpp=BPP):
    """(n, 11, 11) f32 -> {boards: bf16 [NPART, bpp*169+14]};
    board b = r*bpp + j lives in partition r, slot j."""
    import ml_dtypes

    n = boards_core.shape[0]
    P = np.zeros((n, 13, 13), dtype=np.float32)
    P[:, 1:12, 1:12] = boards_core
    P[:, 0, 1:12] = 1
    P[:, 12, 1:12] = 1
    P[:, 1:12, 0] = -1
    P[:, 1:12, 12] = -1
    flat = P.reshape(n // bpp, bpp * 169)
    out = np.zeros((n // bpp, bpp * 169 + 14), dtype=ml_dtypes.bfloat16)
    out[:, : bpp * 169] = flat
    return {"boards": out}


def gather_core(raw8, rawb, sizes=SIZES):
    """raw8: uint8 [NPART, bpp*22*144] (planes 0..21), rawb: bf16
    [NPART, bpp*5*144] (planes 22..26), both plane-major per tile ->
    board-major (NPART*bpp, 27, 144) uint8."""
    bpp = sum(sizes)
    u = np.empty((NPART, bpp, 27, 144), dtype=np.uint8)
    b8 = bb = off = 0
    for s in sizes:
        n8 = 22 * s * 144
        nb = NBF * s * 144
        t8 = raw8[:, b8 : b8 + n8].reshape(NPART, 22, s, 144)
        tb = rawb[:, bb : bb + nb].reshape(NPART, NBF, s, 144)
        u[:, off : off + s, 0:22] = t8.transpose(0, 2, 1, 3)
        u[:, off : off + s, 22:27] = tb.transpose(0, 2, 1, 3).astype(np.uint8)
        b8 += n8
        bb += nb
        off += s
    return u.reshape(NPART * bpp, 27, 144)


def postprocess(u, boards):
    """u: uint8 (B, 27, 144) one-hot from the device; boards (B, 11, 11).
    Applies the 6 wildcard-corner writes the single-idx compare cannot
    represent (3-hot positions), then casts to the output dtype."""
    B = u.shape[0]
    bi = np.arange(B)
    # corner (0,0) -> pos 0: a1=1, a2=-1 are border constants, elem0
    # wildcard => planes {6,15,24}; the compare already set 15.
    u[:, 6, 0] = 1
    u[:, 24, 0] = 1
    # corner (0,11) -> pos 11: a0=1 border, elem1 wildcard, a2=board[0,10]
    # => planes {18+c, 21+c, 24+c}, c = board+1; 21+c already set.
    c = boards[:, 0, 10].astype(np.int64) + 1
    u[bi, 18 + c, 11] = 1
    u[bi, 24 + c, 11] = 1
    # corner (11,0) -> pos 132: a0=-1 border, a1=board[10,0], elem2
    # wildcard => planes {3d, 3d+1, 3d+2}, d = board+1; 3d+1 already set.
    d = boards[:, 10, 0].astype(np.int64) + 1
    u[bi, 3 * d, 132] = 1
    u[bi, 3 * d + 2, 132] = 1
    return u.astype(np.float32).reshape(B, 27, 12, 12)


def run_spmd(nc, in_maps):
    """Like bass2jax.run_bass_via_pjrt, but the donated zero output buffers
    are created ON DEVICE (separate jit) instead of being uploaded from the
    host — avoids a host->device transfer whose tail can overlap and slow
    down kernel execution."""
    import jax
    import jax.numpy as jnp
    from jax.experimental.shard_map import shard_map
    from jax.sharding import Mesh, NamedSharding, PartitionSpec

    import concourse.mybir as mb
    from concourse import bass2jax

    bass2jax.install_neuronx_cc_hook()
    n_cores = len(in_maps)
    partition_name = nc.partition_id_tensor.name if nc.partition_id_tensor else None

    in_names, out_names, out_avals = [], [], []
    for alloc in nc.m.functions[0].allocations:
        if not isinstance(alloc, mb.MemoryLocationSet):
            continue
        name = alloc.memorylocations[0].name
        if alloc.kind == "ExternalInput":
            if name != partition_name:
                in_names.append(name)
        elif alloc.kind == "ExternalOutput":
            out_names.append(name)
            out_avals.append(
                jax.core.ShapedArray(tuple(alloc.tensor_shape), mb.dt.np(alloc.dtype))
            )
    n_params = len(in_names)
    n_outs = len(out_avals)
    all_names = in_names + out_names
    if partition_name is not None:
        all_names.append(partition_name)

    def _body(*args):
        operands = list(args)
        if partition_name is not None:
            operands.append(bass2jax.partition_id_tensor())
        return tuple(
            bass2jax._bass_exec_p.bind(
                *operands,
                out_avals=tuple(out_avals),
                in_names=tuple(all_names),
                out_names=tuple(out_names),
                lowering_input_output_aliases=(),
                sim_require_finite=True,
                sim_require_nnan=True,
                nc=nc,
            )
        )

    devices = jax.devices()[:n_cores]
    mesh = Mesh(np.asarray(devices), ("core",))
    in_specs = (PartitionSpec("core"),) * (n_params + n_outs)
    out_specs = (PartitionSpec("core"),) * n_outs
    sharded = jax.jit(
        shard_map(
            _body, mesh=mesh, in_specs=in_specs, out_specs=out_specs, check_rep=False
        ),
        donate_argnums=tuple(range(n_params, n_params + n_outs)),
        keep_unused=True,
    )
    concat_in = [
        np.concatenate([np.asarray(in_maps[c][k]) for c in range(n_cores)], axis=0)
        for k in in_names
    ]
    # on-device zero buffers (sharded), no host upload
    zero_fn = jax.jit(
        lambda: tuple(
            jnp.zeros((n_cores * a.shape[0], *a.shape[1:]), a.dtype) for a in out_avals
        ),
        out_shardings=tuple(
            NamedSharding(mesh, PartitionSpec("core")) for _ in out_avals
        ),
    )
    zeros = zero_fn()
    out_arrs = sharded(*concat_in, *zeros)
    return [
        {
            k: np.asarray(out_arrs[i]).reshape(n_cores, *out_avals[i].shape)[c]
            for i, k in enumerate(out_names)
        }
        for c in range(n_cores)
    ]


def kernel(boards):
    boards = np.ascontiguousarray(np.asarray(boards), dtype=np.float32)
    assert boards.shape == (BATCH, 11, 11)

    nc = build_nc()
    in_maps = [
        prep_core_input(boards[c * B_CORE : (c + 1) * B_CORE])
        for c in range(N_CORES)
    ]
    results = run_spmd(nc, in_maps)
    u = np.empty((BATCH, 27, 144), dtype=np.uint8)
    for c in range(N_CORES):
        u[c * B_CORE : (c + 1) * B_CORE] = gather_core(
            results[c]["out"], results[c]["outb"]
        )
    return postprocess(u, boards)


# revision 35
# speedup vs baseline: 1.0095x; 1.0095x over previous
"""Trainium2 Bass kernel for the hex-board pattern one-hot encoder.

Reference semantics: boards (B, 11, 11) in {-1,0,1} -> out (B, 27, 12, 12)
f32 where out[b,p,i,j] = 1 iff the 3-tuple (P[i,j], P[i,j+1], P[i+1,j]) of
the border-padded 13x13 board equals pattern p (patterns =
product([-1,0,1], repeat=3)), with wildcard corners at (0,0) [elem0],
(0,11) [elem1], (11,0) [elem2].

Final design, ~73-76us HW exec in clean windows (f32-output v1: ~188us;
u8 board-major v2/v3: ~88us; the device shows degraded windows with
+15% uniform engine-clock inflation — compare configs only against a
same-window control run). Output values are exactly {0, 1}, so the
device computes and writes the one-hot tensor in UINT8 + a small bf16
side tensor (~145 MB total instead of 510 MB of HBM writes; the f32
materialization is a pure dtype cast after the gather). The kernel is
now limited by the output DMA stream (~350 GB/s/core queue capacity)
with compute split across both engines just under that pace:
  - the idx chain runs in bf16 in three DVE ops: a fused
    scalar_tensor_tensor tmp = 3*P[g+1] + P[g+13] (walrus limits STT to
    2D/3D, so the rest is split), jb = 9*P[g] (4x tensor_scalar), and a
    4D tensor_tensor add that also compacts to the 12x12 subgrids; the
    +13 of the classic 0..26 code is folded into the compare constants
    (idx in -13..13),
  - 12 planes on VectorE as (idx == p-13) bf16->u8 tensor_scalar (2x
    two-port mode) and 5 planes as bf16->bf16 (4x packed mode) into a
    separate bf16 output tensor (host casts it during the gather; adding
    a 6th bf16 plane overloads the store queue and nets out slower),
  - 10 planes on ScalarE in ONE op each: u8(Derivative_Erf(idx+13-p)) =
    u8(1.1284*exp(-(idx-(p-13))^2)) which rounds to exactly 1 at equality
    and 0 otherwise (HW-validated; replaces the 2-op Square+Relu pair).
    The tiny LAST tile's ScalarE planes run FIRST, inside the window
    where ScalarE would otherwise idle waiting for idx of tile 0, so
    ~5us of ScalarE work leaves the end of the kernel.

Layout is PLANE-MAJOR per macrotile, in SBUF and HBM ([27, s, 144] per
partition): every plane compare writes one dense 1-free-dim region, and
every plane-chunk store is one contiguous multi-KB run per partition.
(Board-major chunk stores produced 720-864 B strided DMA transfers whose
descriptor overhead halved effective HBM write bandwidth — measured
~200 GB/s/core vs ~340 with contiguous rows.) The host de-transposes
tiles during the u8 gather.

The padded board grid is shipped pre-cast as bf16 (ml_dtypes) so ScalarE
spends no time on int8->f32 casts and the chain needs no ScalarE at all.
Macrotile sizes (16, 14, 2): EVEN sizes only — the DVE 2x two-port mode
silently degrades to 1x when the compare's free size is odd (measured);
the tiny LAST tile bounds the end-of-kernel DMA drain. The idx chain for
tile m+1 runs on DVE before the plane compares of tile m so ScalarE
never stalls on idx.

The 3 wildcard corners need 2 extra ones each at a fixed position whose
value depends only on one board cell (or nothing at all for corner
(0,0)); a single idx value cannot make 3 planes fire, so those 6
fixed-position writes are applied on the host during the u8 gather
(0.15% of output elements).

Pure data parallel across 8 NeuronCores (batch sharding).
"""

import numpy as np

import concourse.bacc as bacc
import concourse.mybir as mybir
from concourse.mybir import AluOpType
from concourse.tile import TileContext

N_CORES = 8
BATCH = 32768
B_CORE = BATCH // N_CORES  # 4096
NPART = 128
BPP = B_CORE // NPART  # 32 boards per partition
SIZES = (16, 14, 2)  # boards/partition per macrotile; sum == BPP; all EVEN
PADW = BPP * 169 + 14  # flat padded boards per partition + shift tail
QF = 27 * 144  # output elems per board

F32 = mybir.dt.float32
BF16 = mybir.dt.bfloat16
U8 = mybir.dt.uint8

# plane split: 12 planes as VectorE is_equal bf16->u8 (2x two-port mode),
# 10 planes as ScalarE Derivative_Erf (1 op each), and 5 planes as VectorE
# is_equal bf16->bf16 (4x packed mode) into a separate bf16 output tensor
# that the host casts during the gather. The bf16 planes cost 2x the HBM
# bytes but nearly halve their DVE time; they are computed FIRST each tile
# so their larger stores issue early instead of in the drain tail.
N_U8 = 15  # planes [0, 15) on VectorE, u8
ACT_PS = list(range(15, 25))  # planes [15, 25) on ScalarE, u8
BF_PS = list(range(25, 27))  # planes [25, 27) on VectorE, bf16
NBF = len(BF_PS)


def build_nc(sizes=SIZES, debug=False):
    bpp = sum(sizes)
    padw = bpp * 169 + 14
    nm = len(sizes)
    offs = [sum(sizes[:i]) for i in range(nm)]

    nc = bacc.Bacc(
        "TRN2", target_bir_lowering=False, debug=debug, enable_partition_id=False
    )

    # board b_local = r*bpp + j (partition-major); input row per partition
    # is the bpp host-padded 169-elem 13x13 grids, pre-cast bf16.
    boards_h = nc.dram_tensor("boards", [NPART, padw], BF16, kind="ExternalInput")
    # HBM outputs are plane-major per tile: u8 [tile][25][s][144] for planes
    # 0..24 and bf16 [tile][2][s][144] for planes 25..26, per partition.
    out_h = nc.dram_tensor(
        "out", [NPART, bpp * 25 * 144], U8, kind="ExternalOutput"
    )
    outb_h = nc.dram_tensor(
        "outb", [NPART, bpp * NBF * 144], BF16, kind="ExternalOutput"
    )

    with TileContext(nc) as tc:
        with (
            # one merged pool for all small tiles (fewer pools -> fewer
            # semaphore ranges for the TileContext teardown to clear)
            tc.tile_pool(name="spool", bufs=2) as spool,
            tc.tile_pool(name="opool", bufs=2) as opool,
        ):
            cpool = pfpool = gpool = ipool = spool
            # per-partition bias constants for the ScalarE dErf (13 - p),
            # built on ScalarE itself via Copy(scale=0, bias=...).
            negp = cpool.tile([NPART, 27], F32, name="negp", bufs=1)

            def negp_init():
                zsrc = nc.const_aps.tensor(0.0, [NPART, 1], F32)
                for p in ACT_PS:
                    nc.scalar.activation(
                        negp[:, p : p + 1], zsrc,
                        mybir.ActivationFunctionType.Copy,
                        bias=float(13 - p), scale=0.0,
                    )

            pf_tiles, idx_tiles = {}, {}

            def fetch(mi):
                if mi < nm and mi not in pf_tiles:
                    s = sizes[mi]
                    w = s * 169 + 14
                    g0 = offs[mi] * 169
                    Pf = pfpool.tile([NPART, w], BF16, name="Pf", bufs=3)
                    nc.sync.dma_start(out=Pf, in_=boards_h[:, g0 : g0 + w])
                    pf_tiles[mi] = Pf

            def chain(mi):
                """idx[mi] = 9*P[g] + 3*P[g+1] + P[g+13] (range -13..13) on
                the 12x12 subgrids, via two fused ops."""
                if mi >= nm or mi in idx_tiles:
                    return
                s = sizes[mi]
                ng = s * 169
                Pf = pf_tiles[mi]
                ib = gpool.tile([NPART, ng], BF16, name="ib")
                jb = gpool.tile([NPART, ng], BF16, name="jb")
                # the last tile's idx lives the whole kernel (its DVE planes
                # drain at the end) — own ring so it never blocks the others
                idx = ipool.tile(
                    [NPART, s, 144], BF16,
                    name="idx2" if mi == nm - 1 else "idx",
                    bufs=1 if mi == nm - 1 else None,
                )
                ibv = ib.rearrange("p (t a b) -> p t a b", a=13, b=13)
                jbv = jb.rearrange("p (t a b) -> p t a b", a=13, b=13)
                idxv4 = idx.rearrange("p t (a b) -> p t a b", a=12, b=12)
                # ib = (P[g+1] * 3) + P[g+13]  (fused; 2D so walrus allows STT)
                nc.vector.scalar_tensor_tensor(
                    ib, Pf[:, 1 : ng + 1], 3.0, Pf[:, 13 : ng + 13],
                    AluOpType.mult, AluOpType.add,
                )
                # jb = 9*P[g] (4x); idx = jb + ib on the 12x12 subgrids (STT
                # rejects 4D inputs, so scale separately then add via 4D TT)
                nc.vector.tensor_scalar(
                    jb, Pf[:, 0:ng], 9.0, None, AluOpType.mult
                )
                nc.vector.tensor_tensor(
                    idxv4, jbv[:, :, 0:12, 0:12], ibv[:, :, 0:12, 0:12],
                    AluOpType.add,
                )
                idx_tiles[mi] = idx

            for mi in range(nm):
                fetch(mi)
            negp_init()
            chain(0)
            chain(nm - 1)  # tiny last tile: its idx is cheap and enables
            # ScalarE to do that tile's planes NOW, inside the window where
            # it would otherwise idle waiting for idx(0) — so ~5us of ACT
            # work leaves the end of the kernel (where the DMA tail chases
            # ScalarE's final store).

            views = {}

            def mk_views(m):
                s = sizes[m]
                nm1 = m == nm - 1
                out_t = opool.tile(
                    [NPART, 25, s, 144], U8,
                    name="out_t2" if nm1 else "out_t", bufs=1 if nm1 else None,
                )
                out_b = opool.tile(
                    [NPART, NBF, s, 144], BF16,
                    name="out_b2" if nm1 else "out_b", bufs=1 if nm1 else None,
                )
                ohv = out_h[:, offs[m] * 25 * 144 :][
                    :, : 25 * s * 144
                ].rearrange("p (q t f) -> p q t f", q=25, t=s, f=144)
                obv = outb_h[:, offs[m] * NBF * 144 :][
                    :, : NBF * s * 144
                ].rearrange("p (q t f) -> p q t f", q=NBF, t=s, f=144)
                views[m] = (out_t, out_b, ohv, obv)
                return views[m]

            def act_planes(m, chunks):
                s = sizes[m]
                out_t, _, ohv, _ = views[m]
                idxf = idx_tiles[m].rearrange("p t f -> p (t f)")
                sf = s * 144
                of = out_t.rearrange("p q t f -> p (q t f)")
                # claim out_t's DMA WAR dep with a tiny multi-wait-capable
                # op; its own dErf overwrites it below.
                c0 = ACT_PS[0]
                nc.scalar.mul(out_t[:, c0, :, 0], out_t[:, c0, :, 0], 0.0)
                for a, b in chunks:
                    for p in range(a, b):
                        nc.scalar.activation(
                            of[:, p * sf : (p + 1) * sf], idxf,
                            mybir.ActivationFunctionType.Derivative_Erf,
                            bias=negp[:, p : p + 1], scale=1.0,
                        )
                    nc.scalar.dma_start(
                        out=ohv[:, a:b, :, :], in_=out_t[:, a:b, :, :]
                    )

            def dve_planes(m):
                s = sizes[m]
                out_t, out_b, ohv, obv = views[m]
                idxf = idx_tiles[m].rearrange("p t f -> p (t f)")
                sf = s * 144
                of = out_t.rearrange("p q t f -> p (q t f)")
                ob = out_b.rearrange("p q t f -> p (q t f)")
                # bf16 planes first (4x mode), one early store for all 5
                for k, p in enumerate(BF_PS):
                    nc.vector.tensor_scalar(
                        ob[:, k * sf : (k + 1) * sf], idxf,
                        float(p - 13), None, AluOpType.is_equal,
                    )
                nc.sync.dma_start(out=obv, in_=out_b)
                # NB: keep the DMA trigger counts/rings exactly as measured
                # best — adding store chunks or moving them between rings
                # measured 3-5us SLOWER via completion-lane reshuffling.
                for a, b in ((0, 8), (8, 15)):
                    for p in range(a, b):
                        nc.vector.tensor_scalar(
                            of[:, p * sf : (p + 1) * sf], idxf,
                            float(p - 13), None, AluOpType.is_equal,
                        )
                    nc.sync.dma_start(
                        out=ohv[:, a:b, :, :], in_=out_t[:, a:b, :, :]
                    )

            # tile2's ScalarE planes run FIRST (during its idx(0) wait)
            mk_views(nm - 1)
            act_planes(nm - 1, ((15, 25),))
            for m in range(nm - 1):
                mk_views(m)
                chain(m + 1) if m + 1 < nm - 1 else None
                dve_planes(m)
                act_planes(m, ((15, 20), (20, 25)))
            # tile2's VectorE planes drain at the end (tiny)
            dve_planes(nm - 1)

    nc.finalize()
    return nc


def prep_core_input(boards_core, bpp=BPP):
    """(n, 11, 11) f32 -> {boards: bf16 [NPART, bpp*169+14]};
    board b = r*bpp + j lives in partition r, slot j."""
    import ml_dtypes

    n = boards_core.shape[0]
    P = np.zeros((n, 13, 13), dtype=np.float32)
    P[:, 1:12, 1:12] = boards_core
    P[:, 0, 1:12] = 1
    P[:, 12, 1:12] = 1
    P[:, 1:12, 0] = -1
    P[:, 1:12, 12] = -1
    flat = P.reshape(n // bpp, bpp * 169)
    out = np.zeros((n // bpp, bpp * 169 + 14), dtype=ml_dtypes.bfloat16)
    out[:, : bpp * 169] = flat
    return {"boards": out}


def gather_core(raw8, rawb, sizes=SIZES):
    """raw8: uint8 [NPART, bpp*25*144] (planes 0..24), rawb: bf16
    [NPART, bpp*2*144] (planes 25..26), both plane-major per tile ->
    board-major (NPART*bpp, 27, 144) uint8."""
    bpp = sum(sizes)
    u = np.empty((NPART, bpp, 27, 144), dtype=np.uint8)
    b8 = bb = off = 0
    for s in sizes:
        n8 = 25 * s * 144
        nb = NBF * s * 144
        t8 = raw8[:, b8 : b8 + n8].reshape(NPART, 25, s, 144)
        tb = rawb[:, bb : bb + nb].reshape(NPART, NBF, s, 144)
        u[:, off : off + s, 0:25] = t8.transpose(0, 2, 1, 3)
        u[:, off : off + s, 25:27] = tb.transpose(0, 2, 1, 3).astype(np.uint8)
        b8 += n8
        bb += nb
        off += s
    return u.reshape(NPART * bpp, 27, 144)


def postprocess(u, boards):
    """u: uint8 (B, 27, 144) one-hot from the device; boards (B, 11, 11).
    Applies the 6 wildcard-corner writes the single-idx compare cannot
    represent (3-hot positions), then casts to the output dtype."""
    B = u.shape[0]
    bi = np.arange(B)
    # corner (0,0) -> pos 0: a1=1, a2=-1 are border constants, elem0
    # wildcard => planes {6,15,24}; the compare already set 15.
    u[:, 6, 0] = 1
    u[:, 24, 0] = 1
    # corner (0,11) -> pos 11: a0=1 border, elem1 wildcard, a2=board[0,10]
    # => planes {18+c, 21+c, 24+c}, c = board+1; 21+c already set.
    c = boards[:, 0, 10].astype(np.int64) + 1
    u[bi, 18 + c, 11] = 1
    u[bi, 24 + c, 11] = 1
    # corner (11,0) -> pos 132: a0=-1 border, a1=board[10,0], elem2
    # wildcard => planes {3d, 3d+1, 3d+2}, d = board+1; 3d+1 already set.
    d = boards[:, 10, 0].astype(np.int64) + 1
    u[bi, 3 * d, 132] = 1
    u[bi, 3 * d + 2, 132] = 1
    return u.astype(np.float32).reshape(B, 27, 12, 12)


def run_spmd(nc, in_maps):
    """Like bass2jax.run_bass_via_pjrt, but the donated zero output buffers
    are created ON DEVICE (separate jit) instead of being uploaded from the
    host — avoids a host->device transfer whose tail can overlap and slow
    down kernel execution."""
    import jax
    import jax.numpy as jnp
    from jax.experimental.shard_map import shard_map
    from jax.sharding import Mesh, NamedSharding, PartitionSpec

    import concourse.mybir as mb
    from concourse import bass2jax

    bass2jax.install_neuronx_cc_hook()
    n_cores = len(in_maps)
    partition_name = nc.partition_id_tensor.name if nc.partition_id_tensor else None

    in_names, out_names, out_avals = [], [], []
    for alloc in nc.m.functions[0].allocations:
        if not isinstance(alloc, mb.MemoryLocationSet):
            continue
        name = alloc.memorylocations[0].name
        if alloc.kind == "ExternalInput":
            if name != partition_name:
                in_names.append(name)
        elif alloc.kind == "ExternalOutput":
            out_names.append(name)
            out_avals.append(
                jax.core.ShapedArray(tuple(alloc.tensor_shape), mb.dt.np(alloc.dtype))
            )
    n_params = len(in_names)
    n_outs = len(out_avals)
    all_names = in_names + out_names
    if partition_name is not None:
        all_names.append(partition_name)

    def _body(*args):
        operands = list(args)
        if partition_name is not None:
            operands.append(bass2jax.partition_id_tensor())
        return tuple(
            bass2jax._bass_exec_p.bind(
                *operands,
                out_avals=tuple(out_avals),
                in_names=tuple(all_names),
                out_names=tuple(out_names),
                lowering_input_output_aliases=(),
                sim_require_finite=True,
                sim_require_nnan=True,
                nc=nc,
            )
        )

    devices = jax.devices()[:n_cores]
    mesh = Mesh(np.asarray(devices), ("core",))
    in_specs = (PartitionSpec("core"),) * (n_params + n_outs)
    out_specs = (PartitionSpec("core"),) * n_outs
    sharded = jax.jit(
        shard_map(
            _body, mesh=mesh, in_specs=in_specs, out_specs=out_specs, check_rep=False
        ),
        donate_argnums=tuple(range(n_params, n_params + n_outs)),
        keep_unused=True,
    )
    concat_in = [
        np.concatenate([np.asarray(in_maps[c][k]) for c in range(n_cores)], axis=0)
        for k in in_names
    ]
    # on-device zero buffers (sharded), no host upload
    zero_fn = jax.jit(
        lambda: tuple(
            jnp.zeros((n_cores * a.shape[0], *a.shape[1:]), a.dtype) for a in out_avals
        ),
        out_shardings=tuple(
            NamedSharding(mesh, PartitionSpec("core")) for _ in out_avals
        ),
    )
    zeros = zero_fn()
    out_arrs = sharded(*concat_in, *zeros)
    return [
        {
            k: np.asarray(out_arrs[i]).reshape(n_cores, *out_avals[i].shape)[c]
            for i, k in enumerate(out_names)
        }
        for c in range(n_cores)
    ]


def kernel(boards):
    boards = np.ascontiguousarray(np.asarray(boards), dtype=np.float32)
    assert boards.shape == (BATCH, 11, 11)

    nc = build_nc()
    in_maps = [
        prep_core_input(boards[c * B_CORE : (c + 1) * B_CORE])
        for c in range(N_CORES)
    ]
    results = run_spmd(nc, in_maps)
    u = np.empty((BATCH, 27, 144), dtype=np.uint8)
    for c in range(N_CORES):
        u[c * B_CORE : (c + 1) * B_CORE] = gather_core(
            results[c]["out"], results[c]["outb"]
        )
    return postprocess(u, boards)
